# revision 53
# baseline (speedup 1.0000x reference)
"""Trainium2 Bass kernel for staircase-sparse varlen GQA attention + paged KV-cache store.

Problem (hardcoded shapes):
  q [8192,16,128] f32, k/v [8192,4,128] f32, k_cache/v_cache [16384,4,128] f32,
  slot_mapping arange(8192) i32, cu_seqlens arange(9)*1024 i32, block_size=128.
Returns (o [8192,2048] f32, k_cache_new, v_cache_new).

Sharding: data-parallel over the B=8 sequences (one per NeuronCore); the
KV-cache scatter + untouched-region copy is sharded over cores as well.

Per-core device kernel (flash-attention style, all matmuls fp16 in / fp32 acc):
  - block-major f32 loads (contiguous 8KB rows -> full HBM bandwidth),
    DVE cast f32->f16, TensorE identity-transpose into [d, token] layout.
  - S^T[kpos, q] = kT_j^T . qT on TensorE, N=512 (4 heads of a kv group).
  - p^T = exp(SCALE * S^T) on ScalarE (PSUM f32 -> SBUF f16), 1024-wide calls.
  - o[q, 0:128] and the softmax denominator o[q, 128] accumulate in one
    matmul: lhsT = p^T, rhs = [v_j | ones] (ones-column trick), PSUM
    accumulation over the staircase j<=i.
  - normalize with DVE reciprocal + tensor_scalar_mul, one [128, 512] DMA
    out per (kv group, row).
  - cache update: DRAM->DRAM DMA copies (touched slots from k/v inputs,
    untouched slots passed through), spread across the kernel.

Emission is interleaved per q-block (load block i, then attention rows i) and
software-pipelined one chunk deep so TensorE/ScalarE/VectorE/DMA overlap from
the first microseconds.
"""

import numpy as np

import concourse.bass as bass
import concourse.mybir as mybir
import concourse.tile as tile
from concourse import bacc, bass_utils
from concourse.masks import make_identity
from concourse.tile import add_dep_helper

# ---- problem constants (hardcoded per harness contract) ----
B, S, H, KV, D = 8, 1024, 16, 4, 128
T = B * S
NUM_SLOTS = 16384
BLOCK = 128
NBLK = S // BLOCK          # 8 staircase blocks per sequence
G = H // KV                # 4 query heads per kv head
SCALE = 0.08838834764831845
N_CORES = 8
UNTOUCHED = NUM_SLOTS - T          # 8192 slots keep their old cache value
UN_PER_CORE = UNTOUCHED // N_CORES  # 1024
VST = 132                  # vb column stride per (kv, j): 128 v cols + 1 ones + pad

F32 = mybir.dt.float32
F16 = mybir.dt.float16


def _emit(nc, tc):
    q_d = nc.dram_tensor("q", [S, H, D], F32, kind="ExternalInput").ap()
    k_d = nc.dram_tensor("k", [S, KV, D], F32, kind="ExternalInput").ap()
    v_d = nc.dram_tensor("v", [S, KV, D], F32, kind="ExternalInput").ap()
    kcu_d = nc.dram_tensor("kc_un", [UN_PER_CORE, KV, D], F32, kind="ExternalInput").ap()
    vcu_d = nc.dram_tensor("vc_un", [UN_PER_CORE, KV, D], F32, kind="ExternalInput").ap()
    o_d = nc.dram_tensor("o", [S, H * D], F32, kind="ExternalOutput").ap()
    kco_d = nc.dram_tensor("kc_out", [S + UN_PER_CORE, KV, D], F32, kind="ExternalOutput").ap()
    vco_d = nc.dram_tensor("vc_out", [S + UN_PER_CORE, KV, D], F32, kind="ExternalOutput").ap()

    # Cache copy jobs for the untouched slots only — the touched region is
    # written straight from the k/v SBUF tiles during the block loads, saving
    # a 4MB/core DRAM re-read. Background jobs go on the (otherwise idle)
    # GpSimd SWDGE queue so they never head-of-line-block the q/k/v loads on
    # the Sync HWDGE queue; the rest are issued on Sync late, after the last
    # input loads.
    cache_jobs = []
    n_chunks = 2
    crows = UN_PER_CORE // n_chunks
    for dst, src in ((kco_d, kcu_d), (vco_d, vcu_d)):
        for c in range(n_chunks):
            cache_jobs.append((dst[S + c * crows: S + (c + 1) * crows],
                               src[c * crows: (c + 1) * crows]))
    gp_jobs, late_jobs = cache_jobs[:2], cache_jobs[2:]

    with (
        tc.tile_pool(name="pers", bufs=1) as pers,
        tc.tile_pool(name="pt", bufs=4) as pt_pool,
        tc.tile_pool(name="osb", bufs=6) as osb_pool,
        tc.tile_pool(name="small", bufs=8) as small_pool,
        tc.tile_pool(name="natq", bufs=8) as natq_pool,
        tc.tile_pool(name="natkv", bufs=4) as natkv_pool,
        tc.tile_pool(name="tp", bufs=2, space="PSUM") as tp_pool,
        tc.tile_pool(name="stp", bufs=2, space="PSUM") as st_pool,
        tc.tile_pool(name="oap", bufs=1, space="PSUM") as oacc_pool,
    ):
        # qT is i-block-major: column index = (i*H + h)*128 + p, so for a fixed
        # q-block i all H heads are contiguous (S^T matmuls span the 4 heads of
        # a kv group, N=512).
        qT = pers.tile([128, H * S], F16, tag="qT")
        kT = pers.tile([128, KV * S], F16, tag="kT")
        vb = pers.tile([128, KV * NBLK * VST], F16, tag="vb")

        # ones columns of vb (position 128 in each VST-stride slot)
        vb3 = vb[:].rearrange("p (n x) -> p n x", x=VST)
        nc.vector.memset(vb3[:, :, D:D + 1], 1.0)

        ident = pers.tile([128, 128], F16, tag="ident")
        make_identity(nc, ident[:])

        def _copy(use_act, out, in_):
            nc.vector.tensor_copy(out, in_)

        def emit_kv_block(blk):
            use_act = False
            rows = slice(blk * BLOCK, (blk + 1) * BLOCK)
            kf32 = natkv_pool.tile([128, KV * D], F32, tag="kf32", name=f"kf32_{blk}")
            nc.sync.dma_start(kf32[:], k_d[rows].rearrange("p c d -> p (c d)"))
            # touched cache slots written straight from SBUF (SWDGE, background)
            nc.gpsimd.dma_start(kco_d[rows].rearrange("p c d -> p (c d)"), kf32[:])
            kf16 = natkv_pool.tile([128, KV * D], F16, tag="kf16", name=f"kf16_{blk}")
            _copy(use_act, kf16[:], kf32[:])
            tpk = tp_pool.tile([128, KV * BLOCK], F16, tag="tp", name=f"tpk_{blk}")
            for kv in range(KV):
                nc.tensor.transpose(tpk[:, kv * BLOCK:(kv + 1) * BLOCK],
                                    kf16[:, kv * D:(kv + 1) * D], ident[:])
            kT4 = kT[:].rearrange("p (c s) -> p c s", c=KV)
            _copy(use_act, kT4[:, :, blk * BLOCK:(blk + 1) * BLOCK],
                  tpk[:].rearrange("p (c b) -> p c b", c=KV))

            vf32 = natkv_pool.tile([128, KV * D], F32, tag="vf32", name=f"vf32_{blk}")
            nc.sync.dma_start(vf32[:], v_d[rows].rearrange("p c d -> p (c d)"))
            nc.gpsimd.dma_start(vco_d[rows].rearrange("p c d -> p (c d)"), vf32[:])
            vb4 = vb[:].rearrange("p (c j x) -> p c j x", c=KV, j=NBLK)
            _copy(use_act, vb4[:, :, blk, 0:D],
                  vf32[:].rearrange("p (c d) -> p c d", c=KV))

        def emit_q_quarter(blk, kv):
            """Load+transpose one kv group's 4 q heads of token block `blk` —
            small units so the first attention rows start within ~5us."""
            use_act = False
            rows = slice(blk * BLOCK, (blk + 1) * BLOCK)
            h0 = kv * G
            qf32 = natq_pool.tile([128, G * D], F32, tag="qf32", name=f"qf32_{blk}_{kv}")
            nc.sync.dma_start(qf32[:],
                              q_d[rows, h0:h0 + G, :].rearrange("p h d -> p (h d)"))
            qf16 = natq_pool.tile([128, G * D], F16, tag="qf16", name=f"qf16_{blk}_{kv}")
            _copy(use_act, qf16[:], qf32[:])
            tpq = tp_pool.tile([128, G * BLOCK], F16, tag="tp", name=f"tpq_{blk}_{kv}")
            for hh in range(G):
                nc.tensor.transpose(tpq[:, hh * BLOCK:(hh + 1) * BLOCK],
                                    qf16[:, hh * D:(hh + 1) * D], ident[:])
            _copy(use_act,
                  qT[:, (blk * H + h0) * BLOCK: (blk * H + h0 + G) * BLOCK],
                  tpq[:])

        def emit_block(blk):
            emit_kv_block(blk)
            for kv in range(KV):
                emit_q_quarter(blk, kv)

        # ---- attention ----
        # The 4 per-head accumulators of a row share ONE 2-bank PSUM tile
        # (offsets 0/129/258 in bank 0, 512 in bank 1). PSUM "pending zero"
        # arms a whole 2KB bank, so only the first matmul into each bank
        # carries start=True and only the last-emitted one carries stop=True;
        # every accumulator's first write still zero-initializes its own bytes.
        OFF = [0, 129, 258, 512]
        ncols = G * BLOCK  # 512 q columns (4 heads) per S^T matmul
        state = {"job": 0, "gp": 0, "pending": None}
        oas_map = {}

        def emit_stage2(chunk, st, pt):
            """exp + PV (+ normalize/store on last chunk of an i-row)."""
            kv, i, jc, jn, first, last = chunk
            h0 = kv * G
            nc.scalar.activation(pt[:, :jn * ncols], st[:, :jn * ncols],
                                 mybir.ActivationFunctionType.Exp, scale=SCALE)
            oa = oas_map[(kv, i)]
            for jj in range(jn):
                j = jc + jj
                vslot = (kv * NBLK + j) * VST
                for h4 in range(G):
                    nc.tensor.matmul(
                        oa[:, OFF[h4]: OFF[h4] + D + 1],
                        lhsT=pt[:, jj * ncols + h4 * BLOCK: jj * ncols + (h4 + 1) * BLOCK],
                        rhs=vb[:, vslot: vslot + D + 1],
                        start=(j == 0 and h4 in (0, 3)),
                        stop=(j == i and h4 in (2, 3)),
                    )
            if last:
                osb = osb_pool.tile([128, G * D], F32, tag="osb", name=f"osb_{kv}_{i}")
                rcp = small_pool.tile([128, 4], F32, tag="rcp", name=f"rcp_{kv}_{i}")
                # denominators sit at cols 128/257/386 (stride 129) and 640
                nc.vector.reciprocal(rcp[:, 0:3], oa[:, D: 3 * (D + 1): D + 1])
                nc.vector.reciprocal(rcp[:, 3:4], oa[:, OFF[3] + D: OFF[3] + D + 1])
                for h4 in range(G):
                    nc.vector.tensor_scalar_mul(osb[:, h4 * D:(h4 + 1) * D],
                                                oa[:, OFF[h4]: OFF[h4] + D],
                                                rcp[:, h4: h4 + 1])
                # early rows store via SWDGE so the Sync HWDGE FIFO stays
                # clear for the latency-critical input loads
                o_eng = nc.gpsimd if i <= 4 else nc.sync
                osb_inst = o_eng.dma_start(
                    o_d[i * BLOCK:(i + 1) * BLOCK, h0 * D:(h0 + G) * D], osb[:])
                del oas_map[(kv, i)]
                if 2 <= i <= 5 and kv == 3 and state["gp"] < len(gp_jobs):
                    # background cache copy on SWDGE, held back until this row
                    # is stored so it can't starve the early input loads
                    dst, src = gp_jobs[state["gp"]]
                    gp_inst = nc.gpsimd.dma_start(dst, src)
                    add_dep_helper(gp_inst.ins, osb_inst.ins, sync=True,
                                   reason="delay background cache copy")
                    state["gp"] += 1
                if state["job"] < len(late_jobs) and i >= 6:
                    dst, src = late_jobs[state["job"]]
                    nc.sync.dma_start(dst, src)
                    state["job"] += 1

        def emit_chunk(chunk):
            kv, i, jc, jn, first, last = chunk
            h0 = kv * G
            if first:
                oas_map[(kv, i)] = oacc_pool.tile([128, 1024], F32, tag="oacc",
                                                  name=f"oacc_kv{kv}_i{i}")
            st = st_pool.tile([128, 1024], F32, tag="st", name=f"st_{kv}_{i}_{jc}")
            pt = pt_pool.tile([128, 1024], F16, tag="pt", name=f"pt_{kv}_{i}_{jc}")
            for jj in range(jn):
                j = jc + jj
                nc.tensor.matmul(
                    st[:, jj * ncols: (jj + 1) * ncols],
                    lhsT=kT[:, kv * S + j * BLOCK: kv * S + (j + 1) * BLOCK],
                    rhs=qT[:, (i * H + h0) * BLOCK: (i * H + h0 + G) * BLOCK],
                    start=True, stop=True,
                )
            if state["pending"] is not None:
                emit_stage2(*state["pending"])
            state["pending"] = (chunk, st, pt)

        # fast start: block 0 quarters interleaved with row-0 chunks
        emit_kv_block(0)
        for kv in range(KV):
            emit_q_quarter(0, kv)
            emit_chunk((kv, 0, 0, 1, True, True))
        emit_block(1)
        emit_kv_block(2)
        for blk in range(1, NBLK):
            i = blk
            if blk + 2 < NBLK:
                emit_kv_block(blk + 2)  # k/v two blocks ahead
            for kv in range(KV):
                # spread next block's q prep between this row's kv groups
                if blk + 1 < NBLK:
                    emit_q_quarter(blk + 1, kv)
                jcs = list(range(0, i + 1, 2))
                for jc in jcs:
                    emit_chunk((kv, i, jc, min(2, i + 1 - jc), jc == 0, jc == jcs[-1]))
        if state["pending"] is not None:
            emit_stage2(*state["pending"])

        # any leftover cache jobs
        while state["job"] < len(late_jobs):
            dst, src = late_jobs[state["job"]]
            nc.sync.dma_start(dst, src)
            state["job"] += 1


_PROG = None


def build_program():
    global _PROG
    if _PROG is None:
        nc = bacc.Bacc("TRN2", target_bir_lowering=False, debug=False,
                       num_devices=N_CORES)
        with tile.TileContext(nc) as tc:
            _emit(nc, tc)
        nc.compile()
        _PROG = nc
    return _PROG


def make_in_maps(q, k, v, k_cache, v_cache):
    in_maps = []
    for c in range(N_CORES):
        sl = slice(c * S, (c + 1) * S)
        un = slice(T + c * UN_PER_CORE, T + (c + 1) * UN_PER_CORE)
        in_maps.append({
            "q": np.ascontiguousarray(q[sl]),
            "k": np.ascontiguousarray(k[sl]),
            "v": np.ascontiguousarray(v[sl]),
            "kc_un": np.ascontiguousarray(k_cache[un]),
            "vc_un": np.ascontiguousarray(v_cache[un]),
        })
    return in_maps


def _gather(results, k_cache, v_cache):
    o = np.concatenate([results[c]["o"] for c in range(N_CORES)], axis=0)
    kc = np.empty((NUM_SLOTS, KV, D), np.float32)
    vc = np.empty((NUM_SLOTS, KV, D), np.float32)
    for c in range(N_CORES):
        kc[c * S:(c + 1) * S] = results[c]["kc_out"][:S]
        vc[c * S:(c + 1) * S] = results[c]["vc_out"][:S]
        kc[T + c * UN_PER_CORE: T + (c + 1) * UN_PER_CORE] = results[c]["kc_out"][S:]
        vc[T + c * UN_PER_CORE: T + (c + 1) * UN_PER_CORE] = results[c]["vc_out"][S:]
    return o, kc, vc


def _numpy_fallback(q, k, v, k_cache, v_cache, slot_mapping, cu_seqlens_q):
    """Exact reference semantics in numpy, used only if inputs deviate from
    the hardcoded fast-path layout."""
    kc = k_cache.copy()
    vc = v_cache.copy()
    kc[slot_mapping] = k
    vc[slot_mapping] = v
    b = cu_seqlens_q.shape[0] - 1
    s = q.shape[0] // b
    qb = q.reshape(b, s, H, D)
    kb = np.repeat(k.reshape(b, s, KV, D), G, axis=2)
    vb_ = np.repeat(v.reshape(b, s, KV, D), G, axis=2)
    blk = np.arange(s) // BLOCK
    mask = blk[:, None] >= blk[None, :]
    scores = np.einsum("bqhd,bkhd->bhqk", qb * SCALE, kb)
    scores = np.where(mask[None, None], scores, np.finfo(np.float32).min)
    scores = scores - scores.max(-1, keepdims=True)
    p = np.exp(scores)
    p = p / p.sum(-1, keepdims=True)
    o = np.einsum("bhqk,bkhd->bqhd", p, vb_).astype(np.float32)
    return o.reshape(b * s, H * D), kc, vc


def kernel(q, k, v, k_cache, v_cache, slot_mapping, cu_seqlens_q, cu_seqlens_k,
           block_size):
    q = np.asarray(q, np.float32)
    k = np.asarray(k, np.float32)
    v = np.asarray(v, np.float32)
    k_cache = np.asarray(k_cache, np.float32)
    v_cache = np.asarray(v_cache, np.float32)
    slot_mapping = np.asarray(slot_mapping)
    cu_seqlens_q = np.asarray(cu_seqlens_q)
    cu_seqlens_k = np.asarray(cu_seqlens_k)

    fast = (
        q.shape == (T, H, D) and k.shape == (T, KV, D) and v.shape == (T, KV, D)
        and k_cache.shape == (NUM_SLOTS, KV, D) and v_cache.shape == (NUM_SLOTS, KV, D)
        and int(np.asarray(block_size)) == BLOCK
        and np.array_equal(slot_mapping, np.arange(T, dtype=slot_mapping.dtype))
        and np.array_equal(cu_seqlens_q, np.arange(B + 1, dtype=cu_seqlens_q.dtype) * S)
        and np.array_equal(cu_seqlens_k, np.arange(B + 1, dtype=cu_seqlens_k.dtype) * S)
    )
    if not fast:
        return _numpy_fallback(q, k, v, k_cache, v_cache, slot_mapping, cu_seqlens_q)

    nc = build_program()
    in_maps = make_in_maps(q, k, v, k_cache, v_cache)
    res = bass_utils.run_bass_kernel_spmd(nc, in_maps, core_ids=list(range(N_CORES)))
    return _gather(res.results, k_cache, v_cache)


if __name__ == "__main__":
    rng = np.random.default_rng(0)
    q = rng.standard_normal((T, H, D), dtype=np.float32)
    k = rng.standard_normal((T, KV, D), dtype=np.float32)
    v = rng.standard_normal((T, KV, D), dtype=np.float32)
    kc = np.zeros((NUM_SLOTS, KV, D), np.float32)
    vc = np.zeros((NUM_SLOTS, KV, D), np.float32)
    sm = np.arange(T, dtype=np.int32)
    cu = np.arange(B + 1, dtype=np.int32) * S
    o, kcn, vcn = kernel(q=q, k=k, v=v, k_cache=kc, v_cache=vc, slot_mapping=sm,
                         cu_seqlens_q=cu, cu_seqlens_k=cu, block_size=128)
    oref, kref, vref = _numpy_fallback(q, k, v, kc, vc, sm, cu)
    print("o relerr:", np.abs(o - oref).max() / np.abs(oref).max())
    print("kc equal:", np.array_equal(kcn, kref), "vc equal:", np.array_equal(vcn, vref))


# revision 54
# speedup vs baseline: 1.0776x; 1.0776x over previous
"""Trainium2 Bass kernel for staircase-sparse varlen GQA attention + paged KV-cache store.

Problem (hardcoded shapes):
  q [8192,16,128] f32, k/v [8192,4,128] f32, k_cache/v_cache [16384,4,128] f32,
  slot_mapping arange(8192) i32, cu_seqlens arange(9)*1024 i32, block_size=128.
Returns (o [8192,2048] f32, k_cache_new, v_cache_new).

Sharding: data-parallel over the B=8 sequences (one per NeuronCore); the
KV-cache scatter + untouched-region copy is sharded over cores as well.

Per-core device kernel (flash-attention style, all matmuls fp16 in / fp32 acc):
  - block-major f32 loads (contiguous 8KB rows -> full HBM bandwidth),
    DVE cast f32->f16, TensorE identity-transpose into [d, token] layout.
  - S^T[kpos, q] = kT_j^T . qT on TensorE, N=512 (4 heads of a kv group).
  - p^T = exp(SCALE * S^T) on ScalarE (PSUM f32 -> SBUF f16), 1024-wide calls.
  - o[q, 0:128] and the softmax denominator o[q, 128] accumulate in one
    matmul: lhsT = p^T, rhs = [v_j | ones] (ones-column trick), PSUM
    accumulation over the staircase j<=i.
  - normalize with DVE reciprocal + tensor_scalar_mul, one [128, 512] DMA
    out per (kv group, row).
  - cache update: DRAM->DRAM DMA copies (touched slots from k/v inputs,
    untouched slots passed through), spread across the kernel.

Emission is interleaved per q-block (load block i, then attention rows i) and
software-pipelined one chunk deep so TensorE/ScalarE/VectorE/DMA overlap from
the first microseconds.
"""

import numpy as np

import concourse.bass as bass
import concourse.mybir as mybir
import concourse.tile as tile
from concourse import bacc, bass_utils
from concourse.masks import make_identity
from concourse.tile import add_dep_helper

# ---- problem constants (hardcoded per harness contract) ----
B, S, H, KV, D = 8, 1024, 16, 4, 128
T = B * S
NUM_SLOTS = 16384
BLOCK = 128
NBLK = S // BLOCK          # 8 staircase blocks per sequence
G = H // KV                # 4 query heads per kv head
SCALE = 0.08838834764831845
N_CORES = 8
UNTOUCHED = NUM_SLOTS - T          # 8192 slots keep their old cache value
UN_PER_CORE = UNTOUCHED // N_CORES  # 1024
VST = 132                  # vb column stride per (kv, j): 128 v cols + 1 ones + pad

F32 = mybir.dt.float32
F16 = mybir.dt.float16


def _emit(nc, tc):
    q_d = nc.dram_tensor("q", [S, H, D], F32, kind="ExternalInput").ap()
    k_d = nc.dram_tensor("k", [S, KV, D], F32, kind="ExternalInput").ap()
    v_d = nc.dram_tensor("v", [S, KV, D], F32, kind="ExternalInput").ap()
    kcu_d = nc.dram_tensor("kc_un", [UN_PER_CORE, KV, D], F32, kind="ExternalInput").ap()
    vcu_d = nc.dram_tensor("vc_un", [UN_PER_CORE, KV, D], F32, kind="ExternalInput").ap()
    o_d = nc.dram_tensor("o", [S, H * D], F32, kind="ExternalOutput").ap()
    kco_d = nc.dram_tensor("kc_out", [S + UN_PER_CORE, KV, D], F32, kind="ExternalOutput").ap()
    vco_d = nc.dram_tensor("vc_out", [S + UN_PER_CORE, KV, D], F32, kind="ExternalOutput").ap()

    # Cache copy jobs for the untouched slots only — the touched region is
    # written straight from the k/v SBUF tiles during the block loads, saving
    # a 4MB/core DRAM re-read. Background jobs go on the (otherwise idle)
    # GpSimd SWDGE queue so they never head-of-line-block the q/k/v loads on
    # the Sync HWDGE queue; the rest are issued on Sync late, after the last
    # input loads.
    cache_jobs = []
    n_chunks = 2
    crows = UN_PER_CORE // n_chunks
    for dst, src in ((kco_d, kcu_d), (vco_d, vcu_d)):
        for c in range(n_chunks):
            cache_jobs.append((dst[S + c * crows: S + (c + 1) * crows],
                               src[c * crows: (c + 1) * crows]))
    gp_jobs, late_jobs = cache_jobs[:2], cache_jobs[2:]

    with (
        tc.tile_pool(name="pers", bufs=1) as pers,
        tc.tile_pool(name="pt", bufs=4) as pt_pool,
        tc.tile_pool(name="osb", bufs=6) as osb_pool,
        tc.tile_pool(name="small", bufs=8) as small_pool,
        tc.tile_pool(name="natq", bufs=8) as natq_pool,
        tc.tile_pool(name="natkv", bufs=4) as natkv_pool,
        tc.tile_pool(name="tp", bufs=2, space="PSUM") as tp_pool,
        tc.tile_pool(name="stp", bufs=2, space="PSUM") as st_pool,
        tc.tile_pool(name="oap", bufs=1, space="PSUM") as oacc_pool,
    ):
        # qT is i-block-major: column index = (i*H + h)*128 + p, so for a fixed
        # q-block i all H heads are contiguous (S^T matmuls span the 4 heads of
        # a kv group, N=512).
        qT = pers.tile([128, H * S], F16, tag="qT")
        kT = pers.tile([128, KV * S], F16, tag="kT")
        vb = pers.tile([128, KV * NBLK * VST], F16, tag="vb")

        # ones columns of vb (position 128 in each VST-stride slot)
        vb3 = vb[:].rearrange("p (n x) -> p n x", x=VST)
        nc.vector.memset(vb3[:, :, D:D + 1], 1.0)

        ident = pers.tile([128, 128], F16, tag="ident")
        make_identity(nc, ident[:])

        def _copy(use_act, out, in_):
            nc.vector.tensor_copy(out, in_)

        def emit_kv_block(blk):
            use_act = False
            rows = slice(blk * BLOCK, (blk + 1) * BLOCK)
            kf32 = natkv_pool.tile([128, KV * D], F32, tag="kf32", name=f"kf32_{blk}")
            nc.sync.dma_start(kf32[:], k_d[rows].rearrange("p c d -> p (c d)"))
            # touched cache slots written straight from SBUF (SWDGE, background)
            nc.gpsimd.dma_start(kco_d[rows].rearrange("p c d -> p (c d)"), kf32[:])
            kf16 = natkv_pool.tile([128, KV * D], F16, tag="kf16", name=f"kf16_{blk}")
            _copy(use_act, kf16[:], kf32[:])
            tpk = tp_pool.tile([128, KV * BLOCK], F16, tag="tp", name=f"tpk_{blk}")
            for kv in range(KV):
                nc.tensor.transpose(tpk[:, kv * BLOCK:(kv + 1) * BLOCK],
                                    kf16[:, kv * D:(kv + 1) * D], ident[:])
            kT4 = kT[:].rearrange("p (c s) -> p c s", c=KV)
            _copy(use_act, kT4[:, :, blk * BLOCK:(blk + 1) * BLOCK],
                  tpk[:].rearrange("p (c b) -> p c b", c=KV))

            vf32 = natkv_pool.tile([128, KV * D], F32, tag="vf32", name=f"vf32_{blk}")
            nc.sync.dma_start(vf32[:], v_d[rows].rearrange("p c d -> p (c d)"))
            nc.gpsimd.dma_start(vco_d[rows].rearrange("p c d -> p (c d)"), vf32[:])
            vb4 = vb[:].rearrange("p (c j x) -> p c j x", c=KV, j=NBLK)
            _copy(use_act, vb4[:, :, blk, 0:D],
                  vf32[:].rearrange("p (c d) -> p c d", c=KV))

        def emit_q_quarter(blk, kv):
            """Load+transpose one kv group's 4 q heads of token block `blk` —
            small units so the first attention rows start within ~5us."""
            use_act = False
            rows = slice(blk * BLOCK, (blk + 1) * BLOCK)
            h0 = kv * G
            qf32 = natq_pool.tile([128, G * D], F32, tag="qf32", name=f"qf32_{blk}_{kv}")
            nc.sync.dma_start(qf32[:],
                              q_d[rows, h0:h0 + G, :].rearrange("p h d -> p (h d)"))
            qf16 = natq_pool.tile([128, G * D], F16, tag="qf16", name=f"qf16_{blk}_{kv}")
            _copy(use_act, qf16[:], qf32[:])
            tpq = tp_pool.tile([128, G * BLOCK], F16, tag="tp", name=f"tpq_{blk}_{kv}")
            for hh in range(G):
                nc.tensor.transpose(tpq[:, hh * BLOCK:(hh + 1) * BLOCK],
                                    qf16[:, hh * D:(hh + 1) * D], ident[:])
            _copy(use_act,
                  qT[:, (blk * H + h0) * BLOCK: (blk * H + h0 + G) * BLOCK],
                  tpq[:])

        def emit_block(blk):
            emit_kv_block(blk)
            for kv in range(KV):
                emit_q_quarter(blk, kv)

        # ---- attention ----
        # The 4 per-head accumulators of a row share ONE 2-bank PSUM tile
        # (offsets 0/129/258 in bank 0, 512 in bank 1). PSUM "pending zero"
        # arms a whole 2KB bank, so only the first matmul into each bank
        # carries start=True and only the last-emitted one carries stop=True;
        # every accumulator's first write still zero-initializes its own bytes.
        OFF = [0, 129, 258, 512]
        ncols = G * BLOCK  # 512 q columns (4 heads) per S^T matmul
        state = {"job": 0, "gp": 0, "pending": None}
        oas_map = {}

        def emit_stage2(chunk, st, pt):
            """exp + PV (+ normalize/store on last chunk of an i-row)."""
            kv, i, jc, jn, first, last = chunk
            h0 = kv * G
            nc.scalar.activation(pt[:, :jn * ncols], st[:, :jn * ncols],
                                 mybir.ActivationFunctionType.Exp, scale=SCALE)
            oa = oas_map[(kv, i)]
            for jj in range(jn):
                j = jc + jj
                vslot = (kv * NBLK + j) * VST
                for h4 in range(G):
                    nc.tensor.matmul(
                        oa[:, OFF[h4]: OFF[h4] + D + 1],
                        lhsT=pt[:, jj * ncols + h4 * BLOCK: jj * ncols + (h4 + 1) * BLOCK],
                        rhs=vb[:, vslot: vslot + D + 1],
                        start=(j == 0 and h4 in (0, 3)),
                        stop=(j == i and h4 in (2, 3)),
                    )
            if last:
                osb = osb_pool.tile([128, G * D], F32, tag="osb", name=f"osb_{kv}_{i}")
                rcp = small_pool.tile([128, 4], F32, tag="rcp", name=f"rcp_{kv}_{i}")
                # denominators sit at cols 128/257/386 (stride 129) and 640
                nc.vector.reciprocal(rcp[:, 0:3], oa[:, D: 3 * (D + 1): D + 1])
                nc.vector.reciprocal(rcp[:, 3:4], oa[:, OFF[3] + D: OFF[3] + D + 1])
                for h4 in range(G):
                    nc.vector.tensor_scalar_mul(osb[:, h4 * D:(h4 + 1) * D],
                                                oa[:, OFF[h4]: OFF[h4] + D],
                                                rcp[:, h4: h4 + 1])
                # early rows store via SWDGE so the Sync HWDGE FIFO stays
                # clear for the latency-critical input loads
                o_eng = nc.gpsimd if i <= 4 else nc.sync
                osb_inst = o_eng.dma_start(
                    o_d[i * BLOCK:(i + 1) * BLOCK, h0 * D:(h0 + G) * D], osb[:])
                del oas_map[(kv, i)]
                if 2 <= i <= 5 and kv == 3 and state["gp"] < len(gp_jobs):
                    # background cache copy on SWDGE, held back until this row
                    # is stored so it can't starve the early input loads
                    dst, src = gp_jobs[state["gp"]]
                    gp_inst = nc.gpsimd.dma_start(dst, src)
                    add_dep_helper(gp_inst.ins, osb_inst.ins, sync=True,
                                   reason="delay background cache copy")
                    state["gp"] += 1
                if state["job"] < len(late_jobs) and i >= 6:
                    dst, src = late_jobs[state["job"]]
                    nc.sync.dma_start(dst, src)
                    state["job"] += 1

        def emit_chunk(chunk):
            kv, i, jc, jn, first, last = chunk
            h0 = kv * G
            if first:
                oas_map[(kv, i)] = oacc_pool.tile([128, 1024], F32, tag="oacc",
                                                  name=f"oacc_kv{kv}_i{i}")
            st = st_pool.tile([128, 1024], F32, tag="st", name=f"st_{kv}_{i}_{jc}")
            pt = pt_pool.tile([128, 1024], F16, tag="pt", name=f"pt_{kv}_{i}_{jc}")
            for jj in range(jn):
                j = jc + jj
                nc.tensor.matmul(
                    st[:, jj * ncols: (jj + 1) * ncols],
                    lhsT=kT[:, kv * S + j * BLOCK: kv * S + (j + 1) * BLOCK],
                    rhs=qT[:, (i * H + h0) * BLOCK: (i * H + h0 + G) * BLOCK],
                    start=True, stop=True,
                )
            if state["pending"] is not None:
                emit_stage2(*state["pending"])
            state["pending"] = (chunk, st, pt)

        # fast start: block 0 quarters interleaved with row-0 chunks
        emit_kv_block(0)
        for kv in range(KV):
            emit_q_quarter(0, kv)
            emit_chunk((kv, 0, 0, 1, True, True))
        emit_block(1)
        for blk in range(1, NBLK):
            i = blk
            if blk + 1 < NBLK:
                emit_kv_block(blk + 1)  # prefetch next block's k/v
            for kv in range(KV):
                # spread next block's q prep between this row's kv groups
                if blk + 1 < NBLK:
                    emit_q_quarter(blk + 1, kv)
                jcs = list(range(0, i + 1, 2))
                for jc in jcs:
                    emit_chunk((kv, i, jc, min(2, i + 1 - jc), jc == 0, jc == jcs[-1]))
        if state["pending"] is not None:
            emit_stage2(*state["pending"])

        # any leftover cache jobs
        while state["job"] < len(late_jobs):
            dst, src = late_jobs[state["job"]]
            nc.sync.dma_start(dst, src)
            state["job"] += 1


_PROG = None


def build_program():
    global _PROG
    if _PROG is None:
        nc = bacc.Bacc("TRN2", target_bir_lowering=False, debug=False,
                       num_devices=N_CORES)
        with tile.TileContext(nc) as tc:
            _emit(nc, tc)
        nc.compile()
        _PROG = nc
    return _PROG


def make_in_maps(q, k, v, k_cache, v_cache):
    in_maps = []
    for c in range(N_CORES):
        sl = slice(c * S, (c + 1) * S)
        un = slice(T + c * UN_PER_CORE, T + (c + 1) * UN_PER_CORE)
        in_maps.append({
            "q": np.ascontiguousarray(q[sl]),
            "k": np.ascontiguousarray(k[sl]),
            "v": np.ascontiguousarray(v[sl]),
            "kc_un": np.ascontiguousarray(k_cache[un]),
            "vc_un": np.ascontiguousarray(v_cache[un]),
        })
    return in_maps


def _gather(results, k_cache, v_cache):
    o = np.concatenate([results[c]["o"] for c in range(N_CORES)], axis=0)
    kc = np.empty((NUM_SLOTS, KV, D), np.float32)
    vc = np.empty((NUM_SLOTS, KV, D), np.float32)
    for c in range(N_CORES):
        kc[c * S:(c + 1) * S] = results[c]["kc_out"][:S]
        vc[c * S:(c + 1) * S] = results[c]["vc_out"][:S]
        kc[T + c * UN_PER_CORE: T + (c + 1) * UN_PER_CORE] = results[c]["kc_out"][S:]
        vc[T + c * UN_PER_CORE: T + (c + 1) * UN_PER_CORE] = results[c]["vc_out"][S:]
    return o, kc, vc


def _numpy_fallback(q, k, v, k_cache, v_cache, slot_mapping, cu_seqlens_q):
    """Exact reference semantics in numpy, used only if inputs deviate from
    the hardcoded fast-path layout."""
    kc = k_cache.copy()
    vc = v_cache.copy()
    kc[slot_mapping] = k
    vc[slot_mapping] = v
    b = cu_seqlens_q.shape[0] - 1
    s = q.shape[0] // b
    qb = q.reshape(b, s, H, D)
    kb = np.repeat(k.reshape(b, s, KV, D), G, axis=2)
    vb_ = np.repeat(v.reshape(b, s, KV, D), G, axis=2)
    blk = np.arange(s) // BLOCK
    mask = blk[:, None] >= blk[None, :]
    scores = np.einsum("bqhd,bkhd->bhqk", qb * SCALE, kb)
    scores = np.where(mask[None, None], scores, np.finfo(np.float32).min)
    scores = scores - scores.max(-1, keepdims=True)
    p = np.exp(scores)
    p = p / p.sum(-1, keepdims=True)
    o = np.einsum("bhqk,bkhd->bqhd", p, vb_).astype(np.float32)
    return o.reshape(b * s, H * D), kc, vc


def kernel(q, k, v, k_cache, v_cache, slot_mapping, cu_seqlens_q, cu_seqlens_k,
           block_size):
    q = np.asarray(q, np.float32)
    k = np.asarray(k, np.float32)
    v = np.asarray(v, np.float32)
    k_cache = np.asarray(k_cache, np.float32)
    v_cache = np.asarray(v_cache, np.float32)
    slot_mapping = np.asarray(slot_mapping)
    cu_seqlens_q = np.asarray(cu_seqlens_q)
    cu_seqlens_k = np.asarray(cu_seqlens_k)

    fast = (
        q.shape == (T, H, D) and k.shape == (T, KV, D) and v.shape == (T, KV, D)
        and k_cache.shape == (NUM_SLOTS, KV, D) and v_cache.shape == (NUM_SLOTS, KV, D)
        and int(np.asarray(block_size)) == BLOCK
        and np.array_equal(slot_mapping, np.arange(T, dtype=slot_mapping.dtype))
        and np.array_equal(cu_seqlens_q, np.arange(B + 1, dtype=cu_seqlens_q.dtype) * S)
        and np.array_equal(cu_seqlens_k, np.arange(B + 1, dtype=cu_seqlens_k.dtype) * S)
    )
    if not fast:
        return _numpy_fallback(q, k, v, k_cache, v_cache, slot_mapping, cu_seqlens_q)

    nc = build_program()
    in_maps = make_in_maps(q, k, v, k_cache, v_cache)
    res = bass_utils.run_bass_kernel_spmd(nc, in_maps, core_ids=list(range(N_CORES)))
    return _gather(res.results, k_cache, v_cache)


if __name__ == "__main__":
    rng = np.random.default_rng(0)
    q = rng.standard_normal((T, H, D), dtype=np.float32)
    k = rng.standard_normal((T, KV, D), dtype=np.float32)
    v = rng.standard_normal((T, KV, D), dtype=np.float32)
    kc = np.zeros((NUM_SLOTS, KV, D), np.float32)
    vc = np.zeros((NUM_SLOTS, KV, D), np.float32)
    sm = np.arange(T, dtype=np.int32)
    cu = np.arange(B + 1, dtype=np.int32) * S
    o, kcn, vcn = kernel(q=q, k=k, v=v, k_cache=kc, v_cache=vc, slot_mapping=sm,
                         cu_seqlens_q=cu, cu_seqlens_k=cu, block_size=128)
    oref, kref, vref = _numpy_fallback(q, k, v, kc, vc, sm, cu)
    print("o relerr:", np.abs(o - oref).max() / np.abs(oref).max())
    print("kc equal:", np.array_equal(kcn, kref), "vc equal:", np.array_equal(vcn, vref))
